# revision 35
# baseline (speedup 1.0000x reference)
"""Trainium2 Bass kernel for nn_Attention_layer (GNN message passing attention).

Math (see harness reference):
  x_Q = [input_x, pe_Q]  (N, 1024);  x_K = [input_x, pe_K]
  Q = x_Q @ WQ[h] + qb;  K = x_K @ WK[h] + kb;  V = input_x @ WV[h] + vb
  attn = softmax(Q K^T / 16, axis=k);  out = concat_h(attn @ V) @ lin_w.T + lin_b

Distribution: 8 NeuronCores, query-dim (N) sharded 512 rows/core; K/V work
replicated (no collectives).  Per core everything is computed in the
"transposed" domain so no on-device transposes are needed:
  - scores^T [k, q] for 4 heads at a time via one 4x-row-tiled matmul burst
    into a [128, 2048] PSUM tile (contraction = head_dim 32)
  - one exp per 4-head group on ScalarE straight out of PSUM (softmax
    max-subtraction is skipped: scores/16 are provably bounded ~+-4.3)
  - P^T @ V via 4x-col-tiled matmuls (M=32) plus ones-vector Z matmuls into a
    double-buffered PSUM tile; VectorE accumulates both into an SBUF f32
    accumulator across the 32 k-chunks (frees PSUM banks for pipelining)
  - K/V projection matmul units are interleaved into the attention stream to
    keep the PE dense (HAM un-throttled)
  - epilogue: gather Z rows with a selector matmul, reciprocal, broadcast
    1/Z with an outer-product matmul, normalize, final linear; host
    transposes the [HID, NQ] per-core output back and concatenates
"""

import os
import sys
import numpy as np
import ml_dtypes

for _p in ("/opt/trn_rl_repo", "/root/.axon_site/_ro/trn_rl_repo"):
    if os.path.isdir(_p) and _p not in sys.path:
        sys.path.insert(0, _p)

N = 4096
IND = 256          # input_x dim
QKD = 1024         # concat dim for Q/K projections
H = 8              # heads
HD = 32            # head dim
HID = 256          # H * HD
NCORES = 8
NQ = N // NCORES   # 512 query rows per core
SCALE = 1.0 / 16.0  # 1/sqrt(HID)

_CACHE = {}


def _build_nc():
    from contextlib import ExitStack
    import concourse.bacc as bacc
    import concourse.tile as tile
    import concourse.mybir as mybir
    from concourse.bass import ds, ts

    f32 = mybir.dt.float32
    bf16 = mybir.dt.bfloat16
    Exp = mybir.ActivationFunctionType.Exp
    mult = mybir.AluOpType.mult
    add = mybir.AluOpType.add

    nc = bacc.Bacc("TRN2", target_bir_lowering=False, debug=False,
                   num_devices=NCORES)

    # ---- DRAM I/O (per-core shards prepared on host) ----
    xkT = nc.dram_tensor("xkT", [QKD, N], bf16, kind="ExternalInput")   # [x;peK]^T
    xqT = nc.dram_tensor("xqT", [QKD, NQ], bf16, kind="ExternalInput")  # [x;peQ]^T rows blk
    wq = nc.dram_tensor("wq", [QKD, HID], bf16, kind="ExternalInput")   # [d,(h,hd)]
    wk = nc.dram_tensor("wk", [QKD, HID], bf16, kind="ExternalInput")
    wv = nc.dram_tensor("wv", [IND, HID], bf16, kind="ExternalInput")
    lwT = nc.dram_tensor("lwT", [HID, HID], bf16, kind="ExternalInput")  # lin_w.T
    bias4 = nc.dram_tensor("bias4", [128, 8], f32, kind="ExternalInput")  # [p, 4m+i]
    out = nc.dram_tensor("out", [HID, NQ], f32, kind="ExternalOutput")   # out^T

    # Z-row gather: pvacc[:, mg, 512:1024] holds Z_{4mg+r} at partition 32r.
    selz_np = np.zeros((128, 2, 8), dtype=np.float32)
    for mg in range(2):
        for r in range(4):
            selz_np[32 * r, mg, 4 * mg + r] = 1.0
    selz_dram = nc.inline_tensor(np.ascontiguousarray(selz_np), name="selz_const")
    # 1/Z broadcast: zbb_m[32j+hd, q] = zr[4m+j, q]
    bsel_np = np.zeros((8, 256), dtype=np.float32)
    for m in range(2):
        for j in range(4):
            bsel_np[4 * m + j, 128 * m + 32 * j:128 * m + 32 * j + 32] = 1.0
    bsel_dram = nc.inline_tensor(bsel_np, name="bsel_const")
    ones_np = np.ones((128, 1), dtype=ml_dtypes.bfloat16)
    ones_dram = nc.inline_tensor(ones_np, name="ones_const")

    with tile.TileContext(nc) as tc, ExitStack() as ctx:
        consts = ctx.enter_context(tc.tile_pool(name="consts", bufs=1))
        big = ctx.enter_context(tc.tile_pool(name="big", bufs=1))
        ptp = ctx.enter_context(tc.tile_pool(name="ptp", bufs=4))
        stp = ctx.enter_context(tc.tile_pool(name="stp", bufs=1, space="PSUM"))

        # ---- SBUF tiles ----
        xkt = big.tile([128, 8, N], bf16, tag="xkt")       # x_K^T  (8 c-chunks)
        xqt = big.tile([128, 8, NQ], bf16, tag="xqt")      # x_Q^T block
        wqt = consts.tile([128, 8, HID], bf16, tag="wqt")
        wkt = consts.tile([128, 8, HID], bf16, tag="wkt")
        wvt = consts.tile([128, 2, HID], bf16, tag="wvt")
        lwt = consts.tile([128, 2, HID], bf16, tag="lwt")
        bt = consts.tile([128, 8], f32, tag="bt")          # [p, 4m+i]
        selz = consts.tile([128, 2, 8], f32, tag="selz")
        bsel = consts.tile([8, 256], f32, tag="bsel")
        ones = consts.tile([128, 1], bf16, tag="ones")

        kt = big.tile([128, 2, N], bf16, tag="kt")         # K^T rows (h,hd)
        qt = big.tile([128, 2, NQ], bf16, tag="qt")        # Q^T
        vt = big.tile([128, 32, HID], bf16, tag="vt")      # V node-major
        # [:, mg, :512] = attn_x^T unnorm (4 heads), [:, mg, 512:] = Z rows
        pvacc = big.tile([128, 2, 2 * NQ], f32, tag="pvacc")
        attn = big.tile([128, 2, NQ], bf16, tag="attn")    # normalized attn_x^T
        zr = big.tile([8, NQ], f32, tag="zr")              # 1/Z per head
        zbbs = big.tile([128, 2, NQ], f32, tag="zbbs")     # 1/Z broadcast
        outsb = big.tile([128, 2, NQ], f32, tag="outsb")

        # ---- const / weight DMAs, ordered by first consumer: the sync
        # engine's transfers drain in issue order, so Q-proj inputs go first
        # and the first K/V tile is split so the narrow units start early ----
        xkT_r = xkT.rearrange("(c p) (n q) -> n p c q", p=128, q=512)
        xqT_r = xqT.rearrange("(c p) q -> p c q", p=128)
        nc.sync.dma_start(wqt[:], wq.rearrange("(c p) o -> p c o", p=128))
        nc.sync.dma_start(bt[:], bias4[:])
        nc.sync.dma_start(xqt[:, :4], xqT_r[:, :4])
        nc.sync.dma_start(xqt[:, 4:], xqT_r[:, 4:])
        nc.sync.dma_start(wkt[:], wk.rearrange("(c p) o -> p c o", p=128))
        nc.sync.dma_start(wvt[:], wv.rearrange("(c p) o -> p c o", p=128))
        nc.sync.dma_start(xkt[:, :, ds(0, 128)], xkT_r[0][:, :, ds(0, 128)])
        nc.sync.dma_start(xkt[:, :, ds(128, 384)], xkT_r[0][:, :, ds(128, 384)])
        nc.sync.dma_start(lwt[:], lwT.rearrange("(c p) o -> p c o", p=128))
        nc.sync.dma_start(selz[:], selz_dram[:])
        nc.sync.dma_start(bsel[:], bsel_dram[:])
        nc.sync.dma_start(ones[:], ones_dram[:])
        for n in range(1, 8):
            nc.sync.dma_start(xkt[:, :, ts(n, 512)], xkT_r[n])

        nc.vector.memset(pvacc[:], 0.0)
        # preload the ACT exp table set while DMAs land (saves ~1.3us off
        # the first real exp's critical path)
        actwarm = consts.tile([8, 16], f32, tag="actwarm")
        nc.vector.memset(actwarm[:], 0.0)
        nc.scalar.activation(actwarm[:], actwarm[:], Exp)

        def k_proj_unit(n, m):
            ps = stp.tile([128, NQ], f32, tag="pz", bufs=4, name=f"kp{n}_{m}")
            for c in range(8):
                nc.tensor.matmul(ps[:, :512], wkt[:, c, ts(m, 128)],
                                 xkt[:, c, ts(n, 512)],
                                 start=(c == 0), stop=(c == 7))
            nc.vector.tensor_scalar_add(kt[:, m, ts(n, 512)], ps[:, :512],
                                        bt[:, 4 * m + 1:4 * m + 2])

        def v_proj_unit(kc):
            ps = stp.tile([128, NQ], f32, tag="pz", bufs=4, name=f"vp{kc}")
            for c in range(2):
                nc.tensor.matmul(ps[:, :HID], xkt[:, c, ds(128 * kc, 128)],
                                 wvt[:, c, :], start=(c == 0), stop=(c == 1))
            nc.vector.tensor_copy(out=vt[:, kc, :], in_=ps[:, :HID])

        # zero the pz PSUM banks once: rows of the Z region outside
        # {0,32,64,96} are never written afterwards and must stay 0.
        for d in range(4):
            dz = stp.tile([128, NQ], f32, tag="pz", bufs=4, name=f"dz{d}")
            nc.vector.memset(dz[:], 0.0)

        def q_proj_unit(m):
            ps = stp.tile([128, NQ], f32, tag="pz", bufs=4, name=f"qp{m}")
            for c in range(8):
                nc.tensor.matmul(ps[:, :NQ], wqt[:, c, ts(m, 128)], xqt[:, c, :],
                                 start=(c == 0), stop=(c == 7))
            nc.vector.tensor_scalar_add(qt[:, m, :], ps[:, :NQ],
                                        bt[:, 4 * m + 0:4 * m + 1])

        def k_proj_narrow(m, lo, w):
            ps = stp.tile([128, NQ], f32, tag="pz", bufs=4,
                          name=f"kn{m}_{lo}")
            for c in range(8):
                nc.tensor.matmul(ps[:, :w], wkt[:, c, ts(m, 128)],
                                 xkt[:, c, ds(lo, w)],
                                 start=(c == 0), stop=(c == 7))
            nc.vector.tensor_scalar_add(kt[:, m, ds(lo, w)], ps[:, :w],
                                        bt[:, 4 * m + 1:4 * m + 2])

        # ---- minimal prologue: just what scores(kc=0, mg=0) needs ----
        q_proj_unit(0)
        k_proj_narrow(0, 0, 128)
        v_proj_unit(0)
        k_proj_narrow(0, 128, 384)

        # two persistent scores tensors (2 banks each): pair A = heads
        # (4mg, 4mg+1), pair B = (4mg+2, 4mg+3).  Separate tensors so a
        # score pair only WAR-waits on its own exp half, keeping ACT gapless.
        stA = stp.tile([128, 2 * NQ], f32, tag="stA", name="stA")
        stB = stp.tile([128, 2 * NQ], f32, tag="stB", name="stB")

        # projection work interleaved into the attention iteration stream:
        # iteration i = 2*kc + mg; node-tile n's projections are emitted
        # during tile n-1's iterations, V chunks ~3 iterations ahead.
        kproj_open = {}

        def k_proj_half(n, m, half):
            if half == 0:
                ps = stp.tile([128, NQ], f32, tag="pz", bufs=4,
                              name=f"kp{n}_{m}")
                kproj_open[(n, m)] = ps
            else:
                ps = kproj_open.pop((n, m))
            for c in range(4 * half, 4 * half + 4):
                nc.tensor.matmul(ps[:, :512], wkt[:, c, ts(m, 128)],
                                 xkt[:, c, ts(n, 512)],
                                 start=(c == 0), stop=(c == 7))
            if half == 1:
                nc.vector.tensor_scalar_add(kt[:, m, ts(n, 512)], ps[:, :512],
                                            bt[:, 4 * m + 1:4 * m + 2])

        pre_work = {0: [lambda: k_proj_unit(0, 1), lambda: q_proj_unit(1)]}
        for n in range(1, 8):

            pre_work.setdefault(8 * n - 7, []).append(
                lambda n=n: k_proj_half(n, 0, 0))
            pre_work.setdefault(8 * n - 6, []).append(
                lambda n=n: k_proj_half(n, 0, 1))
            pre_work.setdefault(8 * n - 4, []).append(
                lambda n=n: k_proj_half(n, 1, 0))
            pre_work.setdefault(8 * n - 3, []).append(
                lambda n=n: k_proj_half(n, 1, 1))
        for kc in range(1, 32):
            pre_work.setdefault(max(1, 2 * kc - 3), []).append(
                lambda kc=kc: v_proj_unit(kc))

        # ---- attention main loop: 64 groups of (k-chunk, 4 heads),
        # PV/Z software-pipelined one group behind the scores/exp ----
        def pvz_unit(pt, kc, mg):
            pvt = stp.tile([128, NQ], f32, tag="pz", bufs=4, name="pvt")
            zt = stp.tile([128, NQ], f32, tag="pz", bufs=4, name="zt")
            for j in range(4):
                h = 4 * mg + j
                nc.tensor.matmul(
                    pvt[ds(32 * j, 32), :],
                    vt[:, kc, ds(32 * h, 32)],
                    pt[:, ts(j, NQ)],
                    start=True, stop=True,
                    tile_position=(0, 32 * j))
            for j in range(4):
                nc.tensor.matmul(
                    zt[ds(32 * j, 1), :],
                    ones[:],
                    pt[:, ts(j, NQ)],
                    start=True, stop=True,
                    tile_position=(0, 32 * j))
            nc.vector.tensor_tensor(pvacc[:, mg, :NQ], pvacc[:, mg, :NQ],
                                    pvt[:], add)
            nc.vector.tensor_tensor(pvacc[:, mg, NQ:], pvacc[:, mg, NQ:],
                                    zt[:], add)

        prev = None
        for i in range(64):
            kc, mg = i // 2, i % 2
            pt = ptp.tile([128, 4 * NQ], bf16, tag="pt", name="pt")
            for half, stH in ((0, stA), (1, stB)):
                for jj in range(2):
                    j = 2 * half + jj
                    nc.tensor.matmul(
                        stH[:, ts(jj, NQ)],
                        kt[ds(32 * j, 32), mg, ds(128 * kc, 128)],
                        qt[ds(32 * j, 32), mg, :],
                        start=True, stop=True,
                        tile_position=(32 * j, 0))
                nc.scalar.activation(pt[:, ds(2 * half * NQ, 2 * NQ)],
                                     stH[:], Exp, scale=SCALE)
            for fn in pre_work.get(i, []):
                fn()
            if prev is not None:
                pvz_unit(*prev)
            prev = (pt, kc, mg)
        pvz_unit(*prev)

        # ---- epilogue ----
        zq = stp.tile([128, NQ], f32, tag="pz", bufs=4, name="zq")
        for mg in range(2):
            nc.tensor.matmul(zq[:8, :NQ], selz[:, mg, :],
                             pvacc[:, mg, NQ:],
                             start=(mg == 0), stop=(mg == 1))
        nc.vector.reciprocal_approx_fast(zr[:], zq[:8, :NQ])
        for m in range(2):
            psb = stp.tile([128, NQ], f32, tag="pz", bufs=4, name=f"zbb{m}")
            nc.tensor.matmul(psb[:, :NQ], bsel[:, ts(m, 128)], zr[:],
                             start=True, stop=True)
            nc.vector.tensor_copy(out=zbbs[:, m, :], in_=psb[:, :NQ])
        for m in range(2):
            nc.vector.tensor_tensor(attn[:, m, :], pvacc[:, m, :NQ],
                                    zbbs[:, m, :], mult)
            nc.vector.tensor_scalar_add(attn[:, m, :], attn[:, m, :],
                                        bt[:, 4 * m + 2:4 * m + 3])
        out_r = out.rearrange("(m p) q -> p m q", p=128)
        for mo in range(2):
            ps = stp.tile([128, NQ], f32, tag="pz", bufs=4, name=f"lin{mo}")
            for c in range(2):
                nc.tensor.matmul(ps[:, :NQ], lwt[:, c, ts(mo, 128)],
                                 attn[:, c, :], start=(c == 0), stop=(c == 1))
            nc.vector.tensor_scalar_add(outsb[:, mo, :], ps[:, :NQ],
                                        bt[:, 4 * mo + 3:4 * mo + 4])
            nc.sync.dma_start(out_r[:, mo], outsb[:, mo, :])

    nc.compile()
    return nc


def _get_nc():
    if "nc" not in _CACHE:
        _CACHE["nc"] = _build_nc()
    return _CACHE["nc"]


def _prep_in_maps(input_x, pe_Q, pe_K, WQ, WK, WV, Q_bias, K_bias, V_bias,
                  lin_w, lin_b):
    bf = ml_dtypes.bfloat16
    x_kT = np.ascontiguousarray(
        np.concatenate([input_x, pe_K], axis=1).T.astype(bf))       # [1024, 4096]
    x_q = np.concatenate([input_x, pe_Q], axis=1)                   # [4096, 1024]
    wq2 = np.ascontiguousarray(
        WQ.transpose(1, 0, 2).reshape(QKD, HID).astype(bf))         # [d,(h,hd)]
    wk2 = np.ascontiguousarray(WK.transpose(1, 0, 2).reshape(QKD, HID).astype(bf))
    wv2 = np.ascontiguousarray(WV.transpose(1, 0, 2).reshape(IND, HID).astype(bf))
    lwTn = np.ascontiguousarray(lin_w.T.astype(bf))                 # [in, out]
    bias4 = np.zeros((128, 8), np.float32)
    for m in range(2):
        for i, vec in enumerate([Q_bias.reshape(HID), K_bias.reshape(HID),
                                 V_bias.reshape(HID), lin_b.reshape(HID)]):
            bias4[:, 4 * m + i] = vec[128 * m:128 * (m + 1)]
    in_maps = []
    for i in range(NCORES):
        xqT_i = np.ascontiguousarray(
            x_q[i * NQ:(i + 1) * NQ].T.astype(bf))                  # [1024, 512]
        in_maps.append({
            "xkT": x_kT, "xqT": xqT_i, "wq": wq2, "wk": wk2, "wv": wv2,
            "lwT": lwTn, "bias4": bias4,
        })
    return in_maps


def _ensure_ntff_hook():
    """The agent image's antenv lacks axon_hooks; synthesize it from the
    boot script's ctypes NTFF implementation so trace=True works."""
    import types
    try:
        from antenv.axon_hooks import get_axon_ntff_profile_hook  # noqa: F401
        return
    except ImportError:
        pass
    sys.path.insert(0, "/root/.axon_site/trn_agent_boot")
    import trn_boot
    hook = trn_boot._ntff_profile_via_ctypes(
        os.environ.get("PJRT_LIBRARY_PATH", "/opt/axon/libaxon_pjrt.so"))
    mod = types.ModuleType("antenv.axon_hooks")
    mod._hook = hook
    mod.get_axon_ntff_profile_hook = lambda: mod._hook
    mod.set_axon_ntff_profile_hook = lambda h: setattr(mod, "_hook", h)
    sys.modules["antenv.axon_hooks"] = mod


def _run(in_maps, trace=False):
    from concourse.bass_utils import run_bass_kernel_spmd
    if trace:
        _ensure_ntff_hook()
    nc = _get_nc()
    res = run_bass_kernel_spmd(nc, in_maps, core_ids=list(range(NCORES)),
                               trace=trace)
    return res


def kernel(input_x, pe_Q, pe_K, A, WQ, WK, WV, Q_bias, K_bias, V_bias,
           lin_w, lin_b):
    in_maps = _prep_in_maps(
        np.asarray(input_x, np.float32), np.asarray(pe_Q, np.float32),
        np.asarray(pe_K, np.float32), np.asarray(WQ, np.float32),
        np.asarray(WK, np.float32), np.asarray(WV, np.float32),
        np.asarray(Q_bias, np.float32), np.asarray(K_bias, np.float32),
        np.asarray(V_bias, np.float32), np.asarray(lin_w, np.float32),
        np.asarray(lin_b, np.float32))
    res = _run(in_maps)
    out_full = np.empty((N, HID), np.float32)
    for i in range(NCORES):
        out_full[i * NQ:(i + 1) * NQ] = res.results[i]["out"].T
    return out_full


def hw_exec_ns(input_x, pe_Q, pe_K, A, WQ, WK, WV, Q_bias, K_bias, V_bias,
               lin_w, lin_b):
    """Run once with NTFF tracing; returns (exec_time_ns, results)."""
    in_maps = _prep_in_maps(
        np.asarray(input_x, np.float32), np.asarray(pe_Q, np.float32),
        np.asarray(pe_K, np.float32), np.asarray(WQ, np.float32),
        np.asarray(WK, np.float32), np.asarray(WV, np.float32),
        np.asarray(Q_bias, np.float32), np.asarray(K_bias, np.float32),
        np.asarray(V_bias, np.float32), np.asarray(lin_w, np.float32),
        np.asarray(lin_b, np.float32))
    res = _run(in_maps, trace=True)
    return res.exec_time_ns, res


# revision 37
# speedup vs baseline: 1.0145x; 1.0145x over previous
"""Trainium2 Bass kernel for nn_Attention_layer (GNN message passing attention).

Math (see harness reference):
  x_Q = [input_x, pe_Q]  (N, 1024);  x_K = [input_x, pe_K]
  Q = x_Q @ WQ[h] + qb;  K = x_K @ WK[h] + kb;  V = input_x @ WV[h] + vb
  attn = softmax(Q K^T / 16, axis=k);  out = concat_h(attn @ V) @ lin_w.T + lin_b

Distribution: 8 NeuronCores, query-dim (N) sharded 512 rows/core; K/V work
replicated (no collectives).  Per core everything is computed in the
"transposed" domain so no on-device transposes are needed:
  - scores^T [k, q] for 4 heads at a time via one 4x-row-tiled matmul burst
    into a [128, 2048] PSUM tile (contraction = head_dim 32)
  - one exp per 4-head group on ScalarE straight out of PSUM (softmax
    max-subtraction is skipped: scores/16 are provably bounded ~+-4.3)
  - P^T @ V via 4x-col-tiled matmuls (M=32) plus ones-vector Z matmuls into a
    double-buffered PSUM tile; VectorE accumulates both into an SBUF f32
    accumulator across the 32 k-chunks (frees PSUM banks for pipelining)
  - K/V projection matmul units are interleaved into the attention stream to
    keep the PE dense (HAM un-throttled)
  - epilogue: gather Z rows with a selector matmul, reciprocal, broadcast
    1/Z with an outer-product matmul, normalize, final linear; host
    transposes the [HID, NQ] per-core output back and concatenates
"""

import os
import sys
import numpy as np
import ml_dtypes

for _p in ("/opt/trn_rl_repo", "/root/.axon_site/_ro/trn_rl_repo"):
    if os.path.isdir(_p) and _p not in sys.path:
        sys.path.insert(0, _p)

N = 4096
IND = 256          # input_x dim
QKD = 1024         # concat dim for Q/K projections
H = 8              # heads
HD = 32            # head dim
HID = 256          # H * HD
NCORES = 8
NQ = N // NCORES   # 512 query rows per core
SCALE = 1.0 / 16.0  # 1/sqrt(HID)

_CACHE = {}


def _build_nc():
    from contextlib import ExitStack
    import concourse.bacc as bacc
    import concourse.tile as tile
    import concourse.mybir as mybir
    from concourse.bass import ds, ts

    f32 = mybir.dt.float32
    bf16 = mybir.dt.bfloat16
    Exp = mybir.ActivationFunctionType.Exp
    mult = mybir.AluOpType.mult
    add = mybir.AluOpType.add

    nc = bacc.Bacc("TRN2", target_bir_lowering=False, debug=False,
                   num_devices=NCORES)

    # ---- DRAM I/O (per-core shards prepared on host) ----
    xkT = nc.dram_tensor("xkT", [QKD, N], bf16, kind="ExternalInput")   # [x;peK]^T
    xqT = nc.dram_tensor("xqT", [QKD, NQ], bf16, kind="ExternalInput")  # [x;peQ]^T rows blk
    wq = nc.dram_tensor("wq", [QKD, HID], bf16, kind="ExternalInput")   # [d,(h,hd)]
    wk = nc.dram_tensor("wk", [QKD, HID], bf16, kind="ExternalInput")
    wv = nc.dram_tensor("wv", [IND, HID], bf16, kind="ExternalInput")
    lwT = nc.dram_tensor("lwT", [HID, HID], bf16, kind="ExternalInput")  # lin_w.T
    bias4 = nc.dram_tensor("bias4", [128, 8], f32, kind="ExternalInput")  # [p, 4m+i]
    out = nc.dram_tensor("out", [HID, NQ], f32, kind="ExternalOutput")   # out^T

    # Z-row gather: pvacc[:, mg, 512:1024] holds Z_{4mg+r} at partition 32r.
    selz_np = np.zeros((128, 2, 8), dtype=np.float32)
    for mg in range(2):
        for r in range(4):
            selz_np[32 * r, mg, 4 * mg + r] = 1.0
    selz_dram = nc.inline_tensor(np.ascontiguousarray(selz_np), name="selz_const")
    # 1/Z broadcast: zbb_m[32j+hd, q] = zr[4m+j, q]
    bsel_np = np.zeros((8, 256), dtype=np.float32)
    for m in range(2):
        for j in range(4):
            bsel_np[4 * m + j, 128 * m + 32 * j:128 * m + 32 * j + 32] = 1.0
    bsel_dram = nc.inline_tensor(bsel_np, name="bsel_const")
    ones_np = np.ones((128, 1), dtype=ml_dtypes.bfloat16)
    ones_dram = nc.inline_tensor(ones_np, name="ones_const")

    with tile.TileContext(nc) as tc, ExitStack() as ctx:
        consts = ctx.enter_context(tc.tile_pool(name="consts", bufs=1))
        big = ctx.enter_context(tc.tile_pool(name="big", bufs=1))
        ptp = ctx.enter_context(tc.tile_pool(name="ptp", bufs=4))
        stp = ctx.enter_context(tc.tile_pool(name="stp", bufs=1, space="PSUM"))

        # ---- SBUF tiles ----
        xkt = big.tile([128, 8, N], bf16, tag="xkt")       # x_K^T  (8 c-chunks)
        xqt = big.tile([128, 8, NQ], bf16, tag="xqt")      # x_Q^T block
        wqt = consts.tile([128, 8, HID], bf16, tag="wqt")
        wkt = consts.tile([128, 8, HID], bf16, tag="wkt")
        wvt = consts.tile([128, 2, HID], bf16, tag="wvt")
        lwt = consts.tile([128, 2, HID], bf16, tag="lwt")
        bt = consts.tile([128, 8], f32, tag="bt")          # [p, 4m+i]
        selz = consts.tile([128, 2, 8], f32, tag="selz")
        bsel = consts.tile([8, 256], f32, tag="bsel")
        ones = consts.tile([128, 1], bf16, tag="ones")

        kt = big.tile([128, 2, N], bf16, tag="kt")         # K^T rows (h,hd)
        qt = big.tile([128, 2, NQ], bf16, tag="qt")        # Q^T
        vt = big.tile([128, 32, HID], bf16, tag="vt")      # V node-major
        # [:, mg, :512] = attn_x^T unnorm (4 heads), [:, mg, 512:] = Z rows
        pvacc = big.tile([128, 2, 2 * NQ], f32, tag="pvacc")
        attn = big.tile([128, 2, NQ], bf16, tag="attn")    # normalized attn_x^T
        zr = big.tile([8, NQ], f32, tag="zr")              # 1/Z per head
        zbbs = big.tile([128, 2, NQ], f32, tag="zbbs")     # 1/Z broadcast
        outsb = big.tile([128, 2, NQ], f32, tag="outsb")

        # ---- const / weight DMAs, ordered by first consumer: the sync
        # engine's transfers drain in issue order, so Q-proj inputs go first
        # and the first K/V tile is split so the narrow units start early ----
        xkT_r = xkT.rearrange("(c p) (n q) -> n p c q", p=128, q=512)
        xqT_r = xqT.rearrange("(c p) q -> p c q", p=128)
        nc.sync.dma_start(wqt[:], wq.rearrange("(c p) o -> p c o", p=128))
        nc.sync.dma_start(bt[:], bias4[:])
        nc.sync.dma_start(xqt[:, :4], xqT_r[:, :4])
        nc.sync.dma_start(xqt[:, 4:], xqT_r[:, 4:])
        nc.sync.dma_start(wkt[:], wk.rearrange("(c p) o -> p c o", p=128))
        nc.sync.dma_start(wvt[:], wv.rearrange("(c p) o -> p c o", p=128))
        nc.sync.dma_start(xkt[:, :, ds(0, 128)], xkT_r[0][:, :, ds(0, 128)])
        nc.sync.dma_start(xkt[:, :, ds(128, 384)], xkT_r[0][:, :, ds(128, 384)])
        nc.sync.dma_start(lwt[:], lwT.rearrange("(c p) o -> p c o", p=128))
        nc.sync.dma_start(selz[:], selz_dram[:])
        nc.sync.dma_start(bsel[:], bsel_dram[:])
        nc.sync.dma_start(ones[:], ones_dram[:])
        for n in range(1, 8):
            nc.sync.dma_start(xkt[:, :, ts(n, 512)], xkT_r[n])

        nc.vector.memset(pvacc[:], 0.0)
        # preload the ACT exp table set while DMAs land (saves ~1.3us off
        # the first real exp's critical path)
        actwarm = consts.tile([8, 16], f32, tag="actwarm")
        nc.vector.memset(actwarm[:], 0.0)
        nc.scalar.activation(actwarm[:], actwarm[:], Exp)

        def k_proj_unit(n, m):
            ps = stp.tile([128, NQ], f32, tag="pz", bufs=4, name=f"kp{n}_{m}")
            for c in range(8):
                nc.tensor.matmul(ps[:, :512], wkt[:, c, ts(m, 128)],
                                 xkt[:, c, ts(n, 512)],
                                 start=(c == 0), stop=(c == 7))
            nc.vector.tensor_scalar_add(kt[:, m, ts(n, 512)], ps[:, :512],
                                        bt[:, 4 * m + 1:4 * m + 2])

        def v_proj_unit(kc):
            ps = stp.tile([128, NQ], f32, tag="pz", bufs=4, name=f"vp{kc}")
            for c in range(2):
                nc.tensor.matmul(ps[:, :HID], xkt[:, c, ds(128 * kc, 128)],
                                 wvt[:, c, :], start=(c == 0), stop=(c == 1))
            nc.vector.tensor_copy(out=vt[:, kc, :], in_=ps[:, :HID])

        # zero the pz PSUM banks once: rows of the Z region outside
        # {0,32,64,96} are never written afterwards and must stay 0.
        for d in range(4):
            dz = stp.tile([128, NQ], f32, tag="pz", bufs=4, name=f"dz{d}")
            nc.vector.memset(dz[:], 0.0)

        def q_proj_unit(m):
            ps = stp.tile([128, NQ], f32, tag="pz", bufs=4, name=f"qp{m}")
            for c in range(8):
                nc.tensor.matmul(ps[:, :NQ], wqt[:, c, ts(m, 128)], xqt[:, c, :],
                                 start=(c == 0), stop=(c == 7))
            nc.vector.tensor_scalar_add(qt[:, m, :], ps[:, :NQ],
                                        bt[:, 4 * m + 0:4 * m + 1])

        def k_proj_narrow(m, lo, w):
            ps = stp.tile([128, NQ], f32, tag="pz", bufs=4,
                          name=f"kn{m}_{lo}")
            for c in range(8):
                nc.tensor.matmul(ps[:, :w], wkt[:, c, ts(m, 128)],
                                 xkt[:, c, ds(lo, w)],
                                 start=(c == 0), stop=(c == 7))
            nc.vector.tensor_scalar_add(kt[:, m, ds(lo, w)], ps[:, :w],
                                        bt[:, 4 * m + 1:4 * m + 2])

        # ---- minimal prologue: just what scores(kc=0, mg=0) needs ----
        q_proj_unit(0)
        k_proj_narrow(0, 0, 128)
        v_proj_unit(0)
        k_proj_narrow(0, 128, 384)

        # two persistent scores tensors (2 banks each): pair A = heads
        # (4mg, 4mg+1), pair B = (4mg+2, 4mg+3).  Separate tensors so a
        # score pair only WAR-waits on its own exp half, keeping ACT gapless.
        stA = stp.tile([128, 2 * NQ], f32, tag="stA", name="stA")
        stB = stp.tile([128, 2 * NQ], f32, tag="stB", name="stB")

        # projection work interleaved into the attention iteration stream:
        # iteration i = 2*kc + mg; node-tile n's projections are emitted
        # during tile n-1's iterations, V chunks ~3 iterations ahead.
        kproj_open = {}

        def k_proj_half(n, m, half):
            if half == 0:
                ps = stp.tile([128, NQ], f32, tag="pz", bufs=4,
                              name=f"kp{n}_{m}")
                kproj_open[(n, m)] = ps
            else:
                ps = kproj_open.pop((n, m))
            for c in range(4 * half, 4 * half + 4):
                nc.tensor.matmul(ps[:, :512], wkt[:, c, ts(m, 128)],
                                 xkt[:, c, ts(n, 512)],
                                 start=(c == 0), stop=(c == 7))
            if half == 1:
                nc.vector.tensor_scalar_add(kt[:, m, ts(n, 512)], ps[:, :512],
                                            bt[:, 4 * m + 1:4 * m + 2])

        pre_work = {0: [lambda: k_proj_narrow(1, 0, 128),
                        lambda: q_proj_unit(1)],
                    1: [lambda: k_proj_narrow(1, 128, 384)]}
        for n in range(1, 8):

            pre_work.setdefault(max(2, 8 * n - 8), []).append(
                lambda n=n: k_proj_half(n, 0, 0))
            pre_work.setdefault(max(3, 8 * n - 6), []).append(
                lambda n=n: k_proj_half(n, 0, 1))
            pre_work.setdefault(8 * n - 4, []).append(
                lambda n=n: k_proj_half(n, 1, 0))
            pre_work.setdefault(8 * n - 2, []).append(
                lambda n=n: k_proj_half(n, 1, 1))
        for kc in range(1, 32):
            pre_work.setdefault(max(1, 2 * kc - 3), []).append(
                lambda kc=kc: v_proj_unit(kc))

        # ---- attention main loop: 64 groups of (k-chunk, 4 heads),
        # PV/Z software-pipelined one group behind the scores/exp ----
        def pvz_unit(pt, kc, mg):
            pvt = stp.tile([128, NQ], f32, tag="pz", bufs=4, name="pvt")
            zt = stp.tile([128, NQ], f32, tag="pz", bufs=4, name="zt")
            for j in range(4):
                h = 4 * mg + j
                nc.tensor.matmul(
                    pvt[ds(32 * j, 32), :],
                    vt[:, kc, ds(32 * h, 32)],
                    pt[:, ts(j, NQ)],
                    start=True, stop=True,
                    tile_position=(0, 32 * j))
            for j in range(4):
                nc.tensor.matmul(
                    zt[ds(32 * j, 1), :],
                    ones[:],
                    pt[:, ts(j, NQ)],
                    start=True, stop=True,
                    tile_position=(0, 32 * j))
            nc.vector.tensor_tensor(pvacc[:, mg, :NQ], pvacc[:, mg, :NQ],
                                    pvt[:], add)
            nc.vector.tensor_tensor(pvacc[:, mg, NQ:], pvacc[:, mg, NQ:],
                                    zt[:], add)

        prev = None
        for i in range(64):
            kc, mg = i // 2, i % 2
            pt = ptp.tile([128, 4 * NQ], bf16, tag="pt", name="pt")
            for half, stH in ((0, stA), (1, stB)):
                for jj in range(2):
                    j = 2 * half + jj
                    nc.tensor.matmul(
                        stH[:, ts(jj, NQ)],
                        kt[ds(32 * j, 32), mg, ds(128 * kc, 128)],
                        qt[ds(32 * j, 32), mg, :],
                        start=True, stop=True,
                        tile_position=(32 * j, 0))
                nc.scalar.activation(pt[:, ds(2 * half * NQ, 2 * NQ)],
                                     stH[:], Exp, scale=SCALE)
            for fn in pre_work.get(i, []):
                fn()
            if prev is not None:
                pvz_unit(*prev)
            prev = (pt, kc, mg)
        pvz_unit(*prev)

        # ---- epilogue ----
        zq = stp.tile([128, NQ], f32, tag="pz", bufs=4, name="zq")
        for mg in range(2):
            nc.tensor.matmul(zq[:8, :NQ], selz[:, mg, :],
                             pvacc[:, mg, NQ:],
                             start=(mg == 0), stop=(mg == 1))
        nc.vector.reciprocal_approx_fast(zr[:], zq[:8, :NQ])
        for m in range(2):
            psb = stp.tile([128, NQ], f32, tag="pz", bufs=4, name=f"zbb{m}")
            nc.tensor.matmul(psb[:, :NQ], bsel[:, ts(m, 128)], zr[:],
                             start=True, stop=True)
            nc.vector.tensor_copy(out=zbbs[:, m, :], in_=psb[:, :NQ])
        for m in range(2):
            nc.vector.tensor_tensor(attn[:, m, :], pvacc[:, m, :NQ],
                                    zbbs[:, m, :], mult)
            nc.vector.tensor_scalar_add(attn[:, m, :], attn[:, m, :],
                                        bt[:, 4 * m + 2:4 * m + 3])
        out_r = out.rearrange("(m p) q -> p m q", p=128)
        for mo in range(2):
            ps = stp.tile([128, NQ], f32, tag="pz", bufs=4, name=f"lin{mo}")
            for c in range(2):
                nc.tensor.matmul(ps[:, :NQ], lwt[:, c, ts(mo, 128)],
                                 attn[:, c, :], start=(c == 0), stop=(c == 1))
            nc.vector.tensor_scalar_add(outsb[:, mo, :], ps[:, :NQ],
                                        bt[:, 4 * mo + 3:4 * mo + 4])
            nc.sync.dma_start(out_r[:, mo], outsb[:, mo, :])

    nc.compile()
    return nc


def _get_nc():
    if "nc" not in _CACHE:
        _CACHE["nc"] = _build_nc()
    return _CACHE["nc"]


def _prep_in_maps(input_x, pe_Q, pe_K, WQ, WK, WV, Q_bias, K_bias, V_bias,
                  lin_w, lin_b):
    bf = ml_dtypes.bfloat16
    x_kT = np.ascontiguousarray(
        np.concatenate([input_x, pe_K], axis=1).T.astype(bf))       # [1024, 4096]
    x_q = np.concatenate([input_x, pe_Q], axis=1)                   # [4096, 1024]
    wq2 = np.ascontiguousarray(
        WQ.transpose(1, 0, 2).reshape(QKD, HID).astype(bf))         # [d,(h,hd)]
    wk2 = np.ascontiguousarray(WK.transpose(1, 0, 2).reshape(QKD, HID).astype(bf))
    wv2 = np.ascontiguousarray(WV.transpose(1, 0, 2).reshape(IND, HID).astype(bf))
    lwTn = np.ascontiguousarray(lin_w.T.astype(bf))                 # [in, out]
    bias4 = np.zeros((128, 8), np.float32)
    for m in range(2):
        for i, vec in enumerate([Q_bias.reshape(HID), K_bias.reshape(HID),
                                 V_bias.reshape(HID), lin_b.reshape(HID)]):
            bias4[:, 4 * m + i] = vec[128 * m:128 * (m + 1)]
    in_maps = []
    for i in range(NCORES):
        xqT_i = np.ascontiguousarray(
            x_q[i * NQ:(i + 1) * NQ].T.astype(bf))                  # [1024, 512]
        in_maps.append({
            "xkT": x_kT, "xqT": xqT_i, "wq": wq2, "wk": wk2, "wv": wv2,
            "lwT": lwTn, "bias4": bias4,
        })
    return in_maps


def _ensure_ntff_hook():
    """The agent image's antenv lacks axon_hooks; synthesize it from the
    boot script's ctypes NTFF implementation so trace=True works."""
    import types
    try:
        from antenv.axon_hooks import get_axon_ntff_profile_hook  # noqa: F401
        return
    except ImportError:
        pass
    sys.path.insert(0, "/root/.axon_site/trn_agent_boot")
    import trn_boot
    hook = trn_boot._ntff_profile_via_ctypes(
        os.environ.get("PJRT_LIBRARY_PATH", "/opt/axon/libaxon_pjrt.so"))
    mod = types.ModuleType("antenv.axon_hooks")
    mod._hook = hook
    mod.get_axon_ntff_profile_hook = lambda: mod._hook
    mod.set_axon_ntff_profile_hook = lambda h: setattr(mod, "_hook", h)
    sys.modules["antenv.axon_hooks"] = mod


def _run(in_maps, trace=False):
    from concourse.bass_utils import run_bass_kernel_spmd
    if trace:
        _ensure_ntff_hook()
    nc = _get_nc()
    res = run_bass_kernel_spmd(nc, in_maps, core_ids=list(range(NCORES)),
                               trace=trace)
    return res


def kernel(input_x, pe_Q, pe_K, A, WQ, WK, WV, Q_bias, K_bias, V_bias,
           lin_w, lin_b):
    in_maps = _prep_in_maps(
        np.asarray(input_x, np.float32), np.asarray(pe_Q, np.float32),
        np.asarray(pe_K, np.float32), np.asarray(WQ, np.float32),
        np.asarray(WK, np.float32), np.asarray(WV, np.float32),
        np.asarray(Q_bias, np.float32), np.asarray(K_bias, np.float32),
        np.asarray(V_bias, np.float32), np.asarray(lin_w, np.float32),
        np.asarray(lin_b, np.float32))
    res = _run(in_maps)
    out_full = np.empty((N, HID), np.float32)
    for i in range(NCORES):
        out_full[i * NQ:(i + 1) * NQ] = res.results[i]["out"].T
    return out_full


def hw_exec_ns(input_x, pe_Q, pe_K, A, WQ, WK, WV, Q_bias, K_bias, V_bias,
               lin_w, lin_b):
    """Run once with NTFF tracing; returns (exec_time_ns, results)."""
    in_maps = _prep_in_maps(
        np.asarray(input_x, np.float32), np.asarray(pe_Q, np.float32),
        np.asarray(pe_K, np.float32), np.asarray(WQ, np.float32),
        np.asarray(WK, np.float32), np.asarray(WV, np.float32),
        np.asarray(Q_bias, np.float32), np.asarray(K_bias, np.float32),
        np.asarray(V_bias, np.float32), np.asarray(lin_w, np.float32),
        np.asarray(lin_b, np.float32))
    res = _run(in_maps, trace=True)
    return res.exec_time_ns, res


# revision 38
# speedup vs baseline: 1.0183x; 1.0038x over previous
"""Trainium2 Bass kernel for nn_Attention_layer (GNN message passing attention).

Math (see harness reference):
  x_Q = [input_x, pe_Q]  (N, 1024);  x_K = [input_x, pe_K]
  Q = x_Q @ WQ[h] + qb;  K = x_K @ WK[h] + kb;  V = input_x @ WV[h] + vb
  attn = softmax(Q K^T / 16, axis=k);  out = concat_h(attn @ V) @ lin_w.T + lin_b

Distribution: 8 NeuronCores, query-dim (N) sharded 512 rows/core; K/V work
replicated (no collectives).  Per core everything is computed in the
"transposed" domain so no on-device transposes are needed:
  - scores^T [k, q] for 4 heads at a time via one 4x-row-tiled matmul burst
    into a [128, 2048] PSUM tile (contraction = head_dim 32)
  - one exp per 4-head group on ScalarE straight out of PSUM (softmax
    max-subtraction is skipped: scores/16 are provably bounded ~+-4.3)
  - P^T @ V via 4x-col-tiled matmuls (M=32) plus ones-vector Z matmuls into a
    double-buffered PSUM tile; VectorE accumulates both into an SBUF f32
    accumulator across the 32 k-chunks (frees PSUM banks for pipelining)
  - K/V projection matmul units are interleaved into the attention stream to
    keep the PE dense (HAM un-throttled)
  - epilogue: gather Z rows with a selector matmul, reciprocal, broadcast
    1/Z with an outer-product matmul, normalize, final linear; host
    transposes the [HID, NQ] per-core output back and concatenates
"""

import os
import sys
import numpy as np
import ml_dtypes

for _p in ("/opt/trn_rl_repo", "/root/.axon_site/_ro/trn_rl_repo"):
    if os.path.isdir(_p) and _p not in sys.path:
        sys.path.insert(0, _p)

N = 4096
IND = 256          # input_x dim
QKD = 1024         # concat dim for Q/K projections
H = 8              # heads
HD = 32            # head dim
HID = 256          # H * HD
NCORES = 8
NQ = N // NCORES   # 512 query rows per core
SCALE = 1.0 / 16.0  # 1/sqrt(HID)

_CACHE = {}


def _build_nc():
    from contextlib import ExitStack
    import concourse.bacc as bacc
    import concourse.tile as tile
    import concourse.mybir as mybir
    from concourse.bass import ds, ts

    f32 = mybir.dt.float32
    bf16 = mybir.dt.bfloat16
    Exp = mybir.ActivationFunctionType.Exp
    mult = mybir.AluOpType.mult
    add = mybir.AluOpType.add

    nc = bacc.Bacc("TRN2", target_bir_lowering=False, debug=False,
                   num_devices=NCORES)

    # ---- DRAM I/O (per-core shards prepared on host) ----
    xkT = nc.dram_tensor("xkT", [QKD, N], bf16, kind="ExternalInput")   # [x;peK]^T
    xqT = nc.dram_tensor("xqT", [QKD, NQ], bf16, kind="ExternalInput")  # [x;peQ]^T rows blk
    wq = nc.dram_tensor("wq", [QKD, HID], bf16, kind="ExternalInput")   # [d,(h,hd)]
    wk = nc.dram_tensor("wk", [QKD, HID], bf16, kind="ExternalInput")
    wv = nc.dram_tensor("wv", [IND, HID], bf16, kind="ExternalInput")
    lwT = nc.dram_tensor("lwT", [HID, HID], bf16, kind="ExternalInput")  # lin_w.T
    bias4 = nc.dram_tensor("bias4", [128, 8], f32, kind="ExternalInput")  # [p, 4m+i]
    out = nc.dram_tensor("out", [HID, NQ], f32, kind="ExternalOutput")   # out^T

    # Z-row gather: pvacc[:, mg, 512:1024] holds Z_{4mg+r} at partition 32r.
    selz_np = np.zeros((128, 2, 8), dtype=np.float32)
    for mg in range(2):
        for r in range(4):
            selz_np[32 * r, mg, 4 * mg + r] = 1.0
    selz_dram = nc.inline_tensor(np.ascontiguousarray(selz_np), name="selz_const")
    # 1/Z broadcast: zbb_m[32j+hd, q] = zr[4m+j, q]
    bsel_np = np.zeros((8, 256), dtype=np.float32)
    for m in range(2):
        for j in range(4):
            bsel_np[4 * m + j, 128 * m + 32 * j:128 * m + 32 * j + 32] = 1.0
    bsel_dram = nc.inline_tensor(bsel_np, name="bsel_const")
    ones_np = np.ones((128, 1), dtype=ml_dtypes.bfloat16)
    ones_dram = nc.inline_tensor(ones_np, name="ones_const")

    with tile.TileContext(nc) as tc, ExitStack() as ctx:
        consts = ctx.enter_context(tc.tile_pool(name="consts", bufs=1))
        big = ctx.enter_context(tc.tile_pool(name="big", bufs=1))
        ptp = ctx.enter_context(tc.tile_pool(name="ptp", bufs=4))
        stp = ctx.enter_context(tc.tile_pool(name="stp", bufs=1, space="PSUM"))

        # ---- SBUF tiles ----
        xkt = big.tile([128, 8, N], bf16, tag="xkt")       # x_K^T  (8 c-chunks)
        xqt = big.tile([128, 8, NQ], bf16, tag="xqt")      # x_Q^T block
        wqt = consts.tile([128, 8, HID], bf16, tag="wqt")
        wkt = consts.tile([128, 8, HID], bf16, tag="wkt")
        wvt = consts.tile([128, 2, HID], bf16, tag="wvt")
        lwt = consts.tile([128, 2, HID], bf16, tag="lwt")
        bt = consts.tile([128, 8], f32, tag="bt")          # [p, 4m+i]
        selz = consts.tile([128, 2, 8], f32, tag="selz")
        bsel = consts.tile([8, 256], f32, tag="bsel")
        ones = consts.tile([128, 1], bf16, tag="ones")

        kt = big.tile([128, 2, N], bf16, tag="kt")         # K^T rows (h,hd)
        qt = big.tile([128, 2, NQ], bf16, tag="qt")        # Q^T
        vt = big.tile([128, 32, HID], bf16, tag="vt")      # V node-major
        # [:, mg, :512] = attn_x^T unnorm (4 heads), [:, mg, 512:] = Z rows
        pvacc = big.tile([128, 2, 2 * NQ], f32, tag="pvacc")
        attn = big.tile([128, 2, NQ], bf16, tag="attn")    # normalized attn_x^T
        zr = big.tile([8, NQ], f32, tag="zr")              # 1/Z per head
        zbbs = big.tile([128, 2, NQ], f32, tag="zbbs")     # 1/Z broadcast
        outsb = big.tile([128, 2, NQ], f32, tag="outsb")

        # ---- const / weight DMAs, ordered by first consumer: the sync
        # engine's transfers drain in issue order, so Q-proj inputs go first
        # and the first K/V tile is split so the narrow units start early ----
        xkT_r = xkT.rearrange("(c p) (n q) -> n p c q", p=128, q=512)
        xqT_r = xqT.rearrange("(c p) q -> p c q", p=128)
        nc.scalar.dma_start(wqt[:], wq.rearrange("(c p) o -> p c o", p=128))
        nc.scalar.dma_start(wkt[:], wk.rearrange("(c p) o -> p c o", p=128))
        nc.scalar.dma_start(xkt[:, :, ds(0, 128)], xkT_r[0][:, :, ds(0, 128)])
        nc.sync.dma_start(bt[:], bias4[:])
        nc.sync.dma_start(xqt[:, :4], xqT_r[:, :4])
        nc.sync.dma_start(xqt[:, 4:], xqT_r[:, 4:])
        nc.sync.dma_start(wvt[:], wv.rearrange("(c p) o -> p c o", p=128))
        nc.sync.dma_start(xkt[:, :, ds(128, 384)], xkT_r[0][:, :, ds(128, 384)])
        nc.sync.dma_start(lwt[:], lwT.rearrange("(c p) o -> p c o", p=128))
        nc.sync.dma_start(selz[:], selz_dram[:])
        nc.sync.dma_start(bsel[:], bsel_dram[:])
        nc.sync.dma_start(ones[:], ones_dram[:])
        for n in range(1, 8):
            nc.sync.dma_start(xkt[:, :, ts(n, 512)], xkT_r[n])

        nc.vector.memset(pvacc[:], 0.0)
        # preload the ACT exp table set while DMAs land (saves ~1.3us off
        # the first real exp's critical path)
        actwarm = consts.tile([8, 16], f32, tag="actwarm")
        nc.vector.memset(actwarm[:], 0.0)
        nc.scalar.activation(actwarm[:], actwarm[:], Exp)

        def k_proj_unit(n, m):
            ps = stp.tile([128, NQ], f32, tag="pz", bufs=4, name=f"kp{n}_{m}")
            for c in range(8):
                nc.tensor.matmul(ps[:, :512], wkt[:, c, ts(m, 128)],
                                 xkt[:, c, ts(n, 512)],
                                 start=(c == 0), stop=(c == 7))
            nc.vector.tensor_scalar_add(kt[:, m, ts(n, 512)], ps[:, :512],
                                        bt[:, 4 * m + 1:4 * m + 2])

        def v_proj_unit(kc):
            ps = stp.tile([128, NQ], f32, tag="pz", bufs=4, name=f"vp{kc}")
            for c in range(2):
                nc.tensor.matmul(ps[:, :HID], xkt[:, c, ds(128 * kc, 128)],
                                 wvt[:, c, :], start=(c == 0), stop=(c == 1))
            nc.vector.tensor_copy(out=vt[:, kc, :], in_=ps[:, :HID])

        # zero the pz PSUM banks once: rows of the Z region outside
        # {0,32,64,96} are never written afterwards and must stay 0.
        for d in range(4):
            dz = stp.tile([128, NQ], f32, tag="pz", bufs=4, name=f"dz{d}")
            nc.vector.memset(dz[:], 0.0)

        def q_proj_unit(m):
            ps = stp.tile([128, NQ], f32, tag="pz", bufs=4, name=f"qp{m}")
            for c in range(8):
                nc.tensor.matmul(ps[:, :NQ], wqt[:, c, ts(m, 128)], xqt[:, c, :],
                                 start=(c == 0), stop=(c == 7))
            nc.vector.tensor_scalar_add(qt[:, m, :], ps[:, :NQ],
                                        bt[:, 4 * m + 0:4 * m + 1])

        def k_proj_narrow(m, lo, w):
            ps = stp.tile([128, NQ], f32, tag="pz", bufs=4,
                          name=f"kn{m}_{lo}")
            for c in range(8):
                nc.tensor.matmul(ps[:, :w], wkt[:, c, ts(m, 128)],
                                 xkt[:, c, ds(lo, w)],
                                 start=(c == 0), stop=(c == 7))
            nc.vector.tensor_scalar_add(kt[:, m, ds(lo, w)], ps[:, :w],
                                        bt[:, 4 * m + 1:4 * m + 2])

        # ---- minimal prologue: just what scores(kc=0, mg=0) needs ----
        q_proj_unit(0)
        k_proj_narrow(0, 0, 128)
        v_proj_unit(0)
        k_proj_narrow(0, 128, 384)

        # two persistent scores tensors (2 banks each): pair A = heads
        # (4mg, 4mg+1), pair B = (4mg+2, 4mg+3).  Separate tensors so a
        # score pair only WAR-waits on its own exp half, keeping ACT gapless.
        stA = stp.tile([128, 2 * NQ], f32, tag="stA", name="stA")
        stB = stp.tile([128, 2 * NQ], f32, tag="stB", name="stB")

        # projection work interleaved into the attention iteration stream:
        # iteration i = 2*kc + mg; node-tile n's projections are emitted
        # during tile n-1's iterations, V chunks ~3 iterations ahead.
        kproj_open = {}

        def k_proj_half(n, m, half):
            if half == 0:
                ps = stp.tile([128, NQ], f32, tag="pz", bufs=4,
                              name=f"kp{n}_{m}")
                kproj_open[(n, m)] = ps
            else:
                ps = kproj_open.pop((n, m))
            for c in range(4 * half, 4 * half + 4):
                nc.tensor.matmul(ps[:, :512], wkt[:, c, ts(m, 128)],
                                 xkt[:, c, ts(n, 512)],
                                 start=(c == 0), stop=(c == 7))
            if half == 1:
                nc.vector.tensor_scalar_add(kt[:, m, ts(n, 512)], ps[:, :512],
                                            bt[:, 4 * m + 1:4 * m + 2])

        pre_work = {0: [lambda: k_proj_narrow(1, 0, 128),
                        lambda: q_proj_unit(1)],
                    1: [lambda: k_proj_narrow(1, 128, 384)]}
        for n in range(1, 8):

            pre_work.setdefault(max(2, 8 * n - 8), []).append(
                lambda n=n: k_proj_half(n, 0, 0))
            pre_work.setdefault(max(3, 8 * n - 6), []).append(
                lambda n=n: k_proj_half(n, 0, 1))
            pre_work.setdefault(8 * n - 4, []).append(
                lambda n=n: k_proj_half(n, 1, 0))
            pre_work.setdefault(8 * n - 2, []).append(
                lambda n=n: k_proj_half(n, 1, 1))
        for kc in range(1, 32):
            pre_work.setdefault(max(1, 2 * kc - 3), []).append(
                lambda kc=kc: v_proj_unit(kc))

        # ---- attention main loop: 64 groups of (k-chunk, 4 heads),
        # PV/Z software-pipelined one group behind the scores/exp ----
        def pvz_unit(pt, kc, mg):
            pvt = stp.tile([128, NQ], f32, tag="pz", bufs=4, name="pvt")
            zt = stp.tile([128, NQ], f32, tag="pz", bufs=4, name="zt")
            for j in range(4):
                h = 4 * mg + j
                nc.tensor.matmul(
                    pvt[ds(32 * j, 32), :],
                    vt[:, kc, ds(32 * h, 32)],
                    pt[:, ts(j, NQ)],
                    start=True, stop=True,
                    tile_position=(0, 32 * j))
            for j in range(4):
                nc.tensor.matmul(
                    zt[ds(32 * j, 1), :],
                    ones[:],
                    pt[:, ts(j, NQ)],
                    start=True, stop=True,
                    tile_position=(0, 32 * j))
            nc.vector.tensor_tensor(pvacc[:, mg, :NQ], pvacc[:, mg, :NQ],
                                    pvt[:], add)
            nc.vector.tensor_tensor(pvacc[:, mg, NQ:], pvacc[:, mg, NQ:],
                                    zt[:], add)

        prev = None
        for i in range(64):
            kc, mg = i // 2, i % 2
            pt = ptp.tile([128, 4 * NQ], bf16, tag="pt", name="pt")
            for half, stH in ((0, stA), (1, stB)):
                for jj in range(2):
                    j = 2 * half + jj
                    nc.tensor.matmul(
                        stH[:, ts(jj, NQ)],
                        kt[ds(32 * j, 32), mg, ds(128 * kc, 128)],
                        qt[ds(32 * j, 32), mg, :],
                        start=True, stop=True,
                        tile_position=(32 * j, 0))
                nc.scalar.activation(pt[:, ds(2 * half * NQ, 2 * NQ)],
                                     stH[:], Exp, scale=SCALE)
            for fn in pre_work.get(i, []):
                fn()
            if prev is not None:
                pvz_unit(*prev)
            prev = (pt, kc, mg)
        pvz_unit(*prev)

        # ---- epilogue ----
        zq = stp.tile([128, NQ], f32, tag="pz", bufs=4, name="zq")
        for mg in range(2):
            nc.tensor.matmul(zq[:8, :NQ], selz[:, mg, :],
                             pvacc[:, mg, NQ:],
                             start=(mg == 0), stop=(mg == 1))
        nc.vector.reciprocal_approx_fast(zr[:], zq[:8, :NQ])
        for m in range(2):
            psb = stp.tile([128, NQ], f32, tag="pz", bufs=4, name=f"zbb{m}")
            nc.tensor.matmul(psb[:, :NQ], bsel[:, ts(m, 128)], zr[:],
                             start=True, stop=True)
            nc.vector.tensor_copy(out=zbbs[:, m, :], in_=psb[:, :NQ])
        for m in range(2):
            nc.vector.tensor_tensor(attn[:, m, :], pvacc[:, m, :NQ],
                                    zbbs[:, m, :], mult)
            nc.vector.tensor_scalar_add(attn[:, m, :], attn[:, m, :],
                                        bt[:, 4 * m + 2:4 * m + 3])
        out_r = out.rearrange("(m p) q -> p m q", p=128)
        for mo in range(2):
            ps = stp.tile([128, NQ], f32, tag="pz", bufs=4, name=f"lin{mo}")
            for c in range(2):
                nc.tensor.matmul(ps[:, :NQ], lwt[:, c, ts(mo, 128)],
                                 attn[:, c, :], start=(c == 0), stop=(c == 1))
            nc.vector.tensor_scalar_add(outsb[:, mo, :], ps[:, :NQ],
                                        bt[:, 4 * mo + 3:4 * mo + 4])
            nc.sync.dma_start(out_r[:, mo], outsb[:, mo, :])

    nc.compile()
    return nc


def _get_nc():
    if "nc" not in _CACHE:
        _CACHE["nc"] = _build_nc()
    return _CACHE["nc"]


def _prep_in_maps(input_x, pe_Q, pe_K, WQ, WK, WV, Q_bias, K_bias, V_bias,
                  lin_w, lin_b):
    bf = ml_dtypes.bfloat16
    x_kT = np.ascontiguousarray(
        np.concatenate([input_x, pe_K], axis=1).T.astype(bf))       # [1024, 4096]
    x_q = np.concatenate([input_x, pe_Q], axis=1)                   # [4096, 1024]
    wq2 = np.ascontiguousarray(
        WQ.transpose(1, 0, 2).reshape(QKD, HID).astype(bf))         # [d,(h,hd)]
    wk2 = np.ascontiguousarray(WK.transpose(1, 0, 2).reshape(QKD, HID).astype(bf))
    wv2 = np.ascontiguousarray(WV.transpose(1, 0, 2).reshape(IND, HID).astype(bf))
    lwTn = np.ascontiguousarray(lin_w.T.astype(bf))                 # [in, out]
    bias4 = np.zeros((128, 8), np.float32)
    for m in range(2):
        for i, vec in enumerate([Q_bias.reshape(HID), K_bias.reshape(HID),
                                 V_bias.reshape(HID), lin_b.reshape(HID)]):
            bias4[:, 4 * m + i] = vec[128 * m:128 * (m + 1)]
    in_maps = []
    for i in range(NCORES):
        xqT_i = np.ascontiguousarray(
            x_q[i * NQ:(i + 1) * NQ].T.astype(bf))                  # [1024, 512]
        in_maps.append({
            "xkT": x_kT, "xqT": xqT_i, "wq": wq2, "wk": wk2, "wv": wv2,
            "lwT": lwTn, "bias4": bias4,
        })
    return in_maps


def _ensure_ntff_hook():
    """The agent image's antenv lacks axon_hooks; synthesize it from the
    boot script's ctypes NTFF implementation so trace=True works."""
    import types
    try:
        from antenv.axon_hooks import get_axon_ntff_profile_hook  # noqa: F401
        return
    except ImportError:
        pass
    sys.path.insert(0, "/root/.axon_site/trn_agent_boot")
    import trn_boot
    hook = trn_boot._ntff_profile_via_ctypes(
        os.environ.get("PJRT_LIBRARY_PATH", "/opt/axon/libaxon_pjrt.so"))
    mod = types.ModuleType("antenv.axon_hooks")
    mod._hook = hook
    mod.get_axon_ntff_profile_hook = lambda: mod._hook
    mod.set_axon_ntff_profile_hook = lambda h: setattr(mod, "_hook", h)
    sys.modules["antenv.axon_hooks"] = mod


def _run(in_maps, trace=False):
    from concourse.bass_utils import run_bass_kernel_spmd
    if trace:
        _ensure_ntff_hook()
    nc = _get_nc()
    res = run_bass_kernel_spmd(nc, in_maps, core_ids=list(range(NCORES)),
                               trace=trace)
    return res


def kernel(input_x, pe_Q, pe_K, A, WQ, WK, WV, Q_bias, K_bias, V_bias,
           lin_w, lin_b):
    in_maps = _prep_in_maps(
        np.asarray(input_x, np.float32), np.asarray(pe_Q, np.float32),
        np.asarray(pe_K, np.float32), np.asarray(WQ, np.float32),
        np.asarray(WK, np.float32), np.asarray(WV, np.float32),
        np.asarray(Q_bias, np.float32), np.asarray(K_bias, np.float32),
        np.asarray(V_bias, np.float32), np.asarray(lin_w, np.float32),
        np.asarray(lin_b, np.float32))
    res = _run(in_maps)
    out_full = np.empty((N, HID), np.float32)
    for i in range(NCORES):
        out_full[i * NQ:(i + 1) * NQ] = res.results[i]["out"].T
    return out_full


def hw_exec_ns(input_x, pe_Q, pe_K, A, WQ, WK, WV, Q_bias, K_bias, V_bias,
               lin_w, lin_b):
    """Run once with NTFF tracing; returns (exec_time_ns, results)."""
    in_maps = _prep_in_maps(
        np.asarray(input_x, np.float32), np.asarray(pe_Q, np.float32),
        np.asarray(pe_K, np.float32), np.asarray(WQ, np.float32),
        np.asarray(WK, np.float32), np.asarray(WV, np.float32),
        np.asarray(Q_bias, np.float32), np.asarray(K_bias, np.float32),
        np.asarray(V_bias, np.float32), np.asarray(lin_w, np.float32),
        np.asarray(lin_b, np.float32))
    res = _run(in_maps, trace=True)
    return res.exec_time_ns, res
